# revision 4
# baseline (speedup 1.0000x reference)
"""DisentangledSelfAttention (DeBERTa-style) Trainium2 Bass kernel.

Sharding: data-parallel over batch B=8 -> one batch element per NeuronCore.
Positional tensors are batch-independent and computed (replicated) per core.

Key algebraic structure exploited:
  rel[i, j] = j - i + 511 depends only on (j - i), and for S=384 only
  rel indices 128..894 (767 values) are ever used.  So:
    Kp_flat[p]  = rel_pos_emb[128 + p] @ Wpk          (p in [0, 767))
    Qp_flat[p]  = rel_pos_emb[128 + p] @ Wpq
    c2p[b,h,i,j] = q[b,i,h] . Kp_flat[j-i+383, h]
    p2c[b,h,i,j] = Qp_flat[j-i+383, h] . k[b,j,h]
  c2p comes from qp[i,p] = q[i].Kp_flat[p]: each 128-row i-tile computes a
  512-wide window of qp, bounces it to DRAM [384x512], and reads the score
  block back with row pitch 511 instead of 512 (a strided "skew" read that
  turns the per-row diagonal shift into a flat 2D access pattern).  p2c
  likewise from kq[j,p'] = k[j].Qp_rev[p'] (Qp rows reversed), read back
  transposed [j, i] with the same skew trick.

bf16 everywhere on the PE: matmul operands are bf16 (1 cycle/row in the
cost model vs fp32's 4 and fp32 transpose's 2), and the DRAM bounce is
bf16, halving the dominant DMA traffic.  The two PE "transposes" are
expressed as REGULAR matmuls against a bf16 identity (transpose-dtype
rules would force fp32): out = lhsT^T @ I.  Softmax normalization is
fused into the weight transpose by using diag(1/rowsum) instead of I,
so normalized-transposed weights come out of the PE directly.

The softmax scale (dh**-0.5) and q_bias/v_bias are folded into the weights
and biases on the host before upload.
"""

import os
import sys

import numpy as np

B, S, D, H = 8, 384, 768, 12
DH = D // H          # 64
MAX_POS = 512
NP = 767             # number of used relative positions (128..894)
SCALE = DH ** -0.5

NIT = S // 128       # 3 i/j tiles
NDT = D // 128       # 6 d tiles
NPP = 768            # positional axis padded to even
NW = 512             # per-i-tile window of the positional axis (511 used)
# chunks of the positional axis (PSUM free dim <= 512 fp32)
PCHUNKS = [(0, 384), (384, 384)]

_CACHE = {}


def _import_concourse():
    try:
        import concourse.bass  # noqa: F401
    except ImportError:
        for p in ("/opt/trn_rl_repo", "/root/.axon_site/_ro/trn_rl_repo"):
            if os.path.isdir(p) and p not in sys.path:
                sys.path.insert(0, p)
        import concourse.bass  # noqa: F401


def _build(mm_dtype_name: str):
    """Build + finalize the per-core Bass program (identical on all cores)."""
    _import_concourse()
    import concourse.bass as bass
    import concourse.bacc as bacc
    import concourse.mybir as mybir
    import concourse.tile as tile
    from concourse.bass import ts
    from concourse.masks import make_identity
    from concourse.tile import add_dep_helper

    f32 = mybir.dt.float32
    sdt = getattr(mybir.dt, mm_dtype_name)
    assert mybir.dt.size(sdt) == 2, "kernel assumes a 16-bit matmul dtype"
    bdt = sdt  # bounce dtype
    ADD = mybir.AluOpType.add
    EXP = mybir.ActivationFunctionType.Exp

    nc = bacc.Bacc("TRN2", target_bir_lowering=False, debug=False)

    # ---------------- DRAM I/O ----------------
    xT = nc.dram_tensor("xT", [D, S], sdt, kind="ExternalInput")
    wq = nc.dram_tensor("wq", [D, D], sdt, kind="ExternalInput")
    wk = nc.dram_tensor("wk", [D, D], sdt, kind="ExternalInput")
    wv = nc.dram_tensor("wv", [D, D], sdt, kind="ExternalInput")
    wpk = nc.dram_tensor("wpk", [D, D], sdt, kind="ExternalInput")
    wpq = nc.dram_tensor("wpq", [D, D], sdt, kind="ExternalInput")
    wo = nc.dram_tensor("wo", [D, D], sdt, kind="ExternalInput")
    bq = nc.dram_tensor("bq", [D], f32, kind="ExternalInput")
    bk = nc.dram_tensor("bk", [D], f32, kind="ExternalInput")
    bv = nc.dram_tensor("bv", [D], f32, kind="ExternalInput")
    bo = nc.dram_tensor("bo", [D], f32, kind="ExternalInput")
    relkT = nc.dram_tensor("relkT", [D, NPP], sdt, kind="ExternalInput")
    out = nc.dram_tensor("out", [S, D], f32, kind="ExternalOutput")

    # per-head DRAM scratch for the skew bounce
    qp_dram = [nc.dram_tensor(f"qp_scratch_{h}", [S, NW], bdt) for h in range(H)]
    kq_dram = [nc.dram_tensor(f"kq_scratch_{h}", [S, NW], bdt) for h in range(H)]

    with tile.TileContext(nc) as tc:
        with (
            tc.tile_pool(name="const", bufs=1) as constp,
            tc.tile_pool(name="big", bufs=1) as bigp,
            tc.tile_pool(name="wpool", bufs=2) as wpool,
            tc.tile_pool(name="psA", bufs=2, space="PSUM") as psA,
            tc.tile_pool(name="psSC", bufs=2, space="PSUM") as psSC,
            tc.tile_pool(name="psWT", bufs=3, space="PSUM") as psWT,
            tc.tile_pool(name="psAV", bufs=1, space="PSUM") as psAV,
        ):
            def psum(tag, shape=None):
                pool = {"ps": psA, "sc": psSC, "wtps": psWT, "avps": psAV}[tag]
                return pool.tile(shape or [128, NW], f32, tag=tag, name=tag)

            # rotate PSUM->SBUF copies across DVE/Act (Pool can't touch PSUM)
            _cp_idx = [0]

            def copy_rot(dst, src):
                if _cp_idx[0] % 2 == 0:
                    nc.vector.tensor_copy(dst, src)
                else:
                    nc.scalar.copy(dst, src)
                _cp_idx[0] += 1

            qT_sb = bigp.tile([128, NDT, S], sdt, tag="qT")
            kT_sb = bigp.tile([128, NDT, S], sdt, tag="kT")
            v_sb = bigp.tile([128, NIT, D], sdt, tag="v")
            KpT_sb = bigp.tile([128, NDT, NPP], sdt, tag="KpT")
            QpTr_sb = bigp.tile([128, NDT, NPP], sdt, tag="QpTr")
            attnT_sb = bigp.tile([128, NDT, S], sdt, tag="attnT")

            # ---------- stage 1+2: projections & positional projections ----
            with tc.tile_pool(name="bigtmp", bufs=2) as bigtmp:
                xT_sb = bigtmp.tile([128, NDT, S], sdt, tag="bigtmp")
                for ko in range(NDT):
                    nc.sync.dma_start(
                        xT_sb[:, ko, :], xT[ts(ko, 128), :]
                    )
                ident = constp.tile([128, 128], sdt, tag="ident")
                make_identity(nc, ident[:])

                bq_sb = constp.tile([128, NDT], f32, tag="bq")
                bk_sb = constp.tile([128, NDT], f32, tag="bk")
                bvf = constp.tile([1, D], f32, tag="bv")
                bof = constp.tile([1, D], f32, tag="bo")
                bvr = constp.tile([128, D], f32, tag="bvr")
                bor = constp.tile([128, D], f32, tag="bor")
                nc.sync.dma_start(bq_sb[:], bq[:].rearrange("(o p) -> p o", p=128))
                nc.sync.dma_start(bk_sb[:], bk[:].rearrange("(o p) -> p o", p=128))
                nc.sync.dma_start(bvf[:], bv[:].unsqueeze(0))
                nc.sync.dma_start(bof[:], bo[:].unsqueeze(0))
                nc.gpsimd.partition_broadcast(bvr[:], bvf[:])
                nc.gpsimd.partition_broadcast(bor[:], bof[:])

                # q^T and k^T : [dout(part), i]  (bias per-partition)
                for wdram, bias_sb, dst in ((wq, bq_sb, qT_sb), (wk, bk_sb, kT_sb)):
                    w_sb = wpool.tile([128, NDT, D], sdt, tag="w")
                    for ko in range(NDT):
                        nc.sync.dma_start(w_sb[:, ko, :], wdram[ts(ko, 128), :])
                    for mo in range(NDT):
                        ps_t = psum("ps")
                        for ko in range(NDT):
                            nc.tensor.matmul(
                                ps_t[:, :S],
                                w_sb[:, ko, ts(mo, 128)],
                                xT_sb[:, ko, :],
                                start=(ko == 0),
                                stop=(ko == NDT - 1),
                            )
                        nc.vector.tensor_scalar_add(
                            dst[:, mo, :], ps_t[:, :S], bias_sb[:, mo : mo + 1]
                        )

                # v : [i(part), dout]  (bias along free dim)
                w_sb = wpool.tile([128, NDT, D], sdt, tag="w")
                for ko in range(NDT):
                    nc.sync.dma_start(w_sb[:, ko, :], wv[ts(ko, 128), :])
                for io in range(NIT):
                    for no in range(2):
                        ps_t = psum("ps")
                        for ko in range(NDT):
                            nc.tensor.matmul(
                                ps_t[:, :384],
                                xT_sb[:, ko, ts(io, 128)],
                                w_sb[:, ko, ts(no, 384)],
                                start=(ko == 0),
                                stop=(ko == NDT - 1),
                            )
                        nc.vector.tensor_tensor(
                            v_sb[:, io, ts(no, 384)],
                            ps_t[:, :384],
                            bvr[:, ts(no, 384)],
                            ADD,
                        )

                # Kp^T and QpRev^T : [dout(part), p].  The reversed rel
                # operand is built on-chip from the forward copy (a DVE copy
                # with a negative-step AP) instead of a second 2.4MB upload.
                relk_keep = None
                for idx, (wdram, dst) in enumerate(
                    ((wpk, KpT_sb), (wpq, QpTr_sb))
                ):
                    w_sb = wpool.tile([128, NDT, D], sdt, tag="w")
                    for ko in range(NDT):
                        nc.sync.dma_start(w_sb[:, ko, :], wdram[ts(ko, 128), :])
                    rel_sb = bigtmp.tile(
                        [128, NDT, NPP], sdt, tag="bigtmp", name=f"rel{idx}"
                    )
                    if idx == 0:
                        for ko in range(NDT):
                            nc.sync.dma_start(
                                rel_sb[:, ko, :], relkT[ts(ko, 128), :]
                            )
                        relk_keep = rel_sb
                    else:
                        # rel_rev[p'] = rel_fwd[766 - p']; col 767 copied
                        # from the forward tile's zero pad
                        nc.vector.tensor_copy(
                            rel_sb[:, :, NPP - 1 : NPP],
                            relk_keep[:, :, NPP - 1 : NPP],
                        )
                        for ko in range(NDT):
                            fwd = relk_keep[:, ko, 0 : NPP - 1]
                            rev = bass.AP(
                                fwd.tensor,
                                fwd.offset + (NPP - 2),
                                [[fwd.ap[0][0], 128], [-1, NPP - 1]],
                            )
                            nc.vector.tensor_copy(
                                rel_sb[:, ko, 0 : NPP - 1], rev
                            )
                    for mo in range(NDT):
                        for ci, (cs, csz) in enumerate(PCHUNKS):
                            ps_t = psum("ps")
                            for ko in range(NDT):
                                nc.tensor.matmul(
                                    ps_t[:, :csz],
                                    w_sb[:, ko, ts(mo, 128)],
                                    rel_sb[:, ko, cs : cs + csz],
                                    start=(ko == 0),
                                    stop=(ko == NDT - 1),
                                )
                            copy_rot(dst[:, mo, cs : cs + csz], ps_t[:, :csz])

                # prefetch Wo now: its bf16 1.1MB fills the stage2->3 DMA
                # lull instead of serializing into the kernel tail
                wo_sb = wpool.tile([128, NDT, D], sdt, tag="w", name="wo_sb")
                for ko in range(NDT):
                    nc.sync.dma_start(wo_sb[:, ko, :], wo[ts(ko, 128), :])

            # ---------- stages 3-5: attention per head ---------------------
            with (
                tc.tile_pool(name="work", bufs=3) as workp,
                tc.tile_pool(name="small", bufs=4) as smallp,
            ):
                import concourse.bass as bass_mod

                qp_w = [None] * H
                kq_w = [None] * H

                def head_slices(h):
                    hp = 64 * (h % 2)
                    ho = h // 2
                    return hp, ho

                def stage3(pair):
                    """qp/kq windowed matmuls + bounce to DRAM for heads 2p, 2p+1.

                    For i-tile t only positional columns [256-128t, 768-128t)
                    are ever read back, so each row tile computes a 512-wide
                    window and the bounce rows are stored with pitch 512.
                    """
                    for which in range(2):  # 0 -> qp, 1 -> kq
                        sbs = {}
                        for sub in range(2):
                            sbs[sub] = workp.tile(
                                [128, NIT, NW], bdt,
                                tag=f"bounce{which}", name=f"bounce{which}", bufs=3,
                            )
                        for it in range(NIT):
                            w0 = 256 - 128 * it
                            for sub in range(2):
                                h = 2 * pair + sub
                                hp, ho = head_slices(h)
                                lhsT = (qT_sb if which == 0 else kT_sb)[
                                    hp : hp + 64, ho, ts(it, 128)
                                ]
                                rhs = (KpT_sb if which == 0 else QpTr_sb)[
                                    hp : hp + 64, ho, w0 : w0 + NW
                                ]
                                ps_t = psum("ps")
                                nc.tensor.matmul(
                                    ps_t[:], lhsT, rhs, start=True, stop=True
                                )
                                copy_rot(sbs[sub][:, it, :], ps_t[:])
                        for sub in range(2):
                            h = 2 * pair + sub
                            dram = (qp_dram if which == 0 else kq_dram)[h]
                            w_inst = nc.sync.dma_start(
                                dram[:].rearrange("(o p) c -> p o c", p=128),
                                sbs[sub][:],
                            )
                            if which == 0:
                                qp_w[h] = w_inst
                            else:
                                kq_w[h] = w_inst

                def stage45(pair):
                    for sub in range(2):
                        h = 2 * pair + sub
                        hp, ho = head_slices(h)
                        wT_sb = workp.tile([128, NIT, S], sdt, tag="wT")
                        # combined skew reads: c2p[t][ip, jf] and p2cT[t][u][jp, if]
                        # flat addr in [384, 512]: 127 + 511*row + 65536*tile + col
                        c2p_sb = workp.tile([128, NIT, S], bdt, tag="c2p", bufs=3)
                        r1 = nc.sync.dma_start(
                            c2p_sb[:],
                            bass_mod.AP(
                                qp_dram[h], 127,
                                [[511, 128], [128 * NW, NIT], [1, S]],
                            ),
                        )
                        add_dep_helper(r1.ins, qp_w[h].ins, reason="qp bounce")
                        # p2cT[u][jp, i] = kq[128u+jp, i-(128u+jp)+383]: the
                        # (t, i%128) dims merge into one contiguous 384-run
                        p2ct_sb = workp.tile(
                            [128, NIT, S], bdt, tag="p2ct", bufs=3
                        )
                        r2 = nc.sync.dma_start(
                            p2ct_sb[:],
                            bass_mod.AP(
                                kq_dram[h], 127,
                                [[511, 128], [128 * NW, NIT], [1, S]],
                            ),
                        )
                        add_dep_helper(r2.ins, kq_w[h].ins, reason="kq bounce")
                        wt_ps = [psum("wtps", shape=[128, S]) for _ in range(NIT)]
                        for t in range(NIT):
                            # ---- scores psum: c2c + p2c via identity-matmul
                            # "transposes" (regular bf16 matmuls accumulate
                            # f32 PSUM; is_transpose would force fp32 operands)
                            sc_ps = psum("sc")
                            nc.tensor.matmul(
                                sc_ps[:, :S],
                                qT_sb[hp : hp + 64, ho, ts(t, 128)],
                                kT_sb[hp : hp + 64, ho, :],
                                start=True,
                                stop=False,
                                skip_group_check=True,
                            )
                            for u in range(NIT):
                                nc.tensor.matmul(
                                    sc_ps[:, ts(u, 128)],
                                    p2ct_sb[:, u, ts(t, 128)],
                                    ident[:],
                                    start=False,
                                    stop=(u == NIT - 1),
                                    skip_group_check=True,
                                )
                            exp_sb = workp.tile([128, S], sdt, tag="exp", bufs=4)
                            nc.vector.tensor_tensor(
                                exp_sb[:], sc_ps[:, :S], c2p_sb[:, t, :], ADD
                            )
                            ssum = smallp.tile([128, 1], f32, tag="ssum")
                            sinv = smallp.tile([128, 1], f32, tag="sinv")
                            nc.scalar.activation(
                                exp_sb[:], exp_sb[:], EXP, accum_out=ssum[:]
                            )
                            nc.vector.reciprocal(sinv[:], ssum[:])
                            # diag(sinv): normalization fuses into the
                            # transpose matmul below (exp^T @ diag)
                            diag = smallp.tile([128, 128], sdt, tag="diag")
                            nc.gpsimd.tensor_scalar_mul(
                                diag[:], ident[:], sinv[:, 0:1]
                            )
                            for u in range(NIT):
                                nc.tensor.matmul(
                                    wt_ps[u][:, ts(t, 128)],
                                    exp_sb[:, ts(u, 128)],
                                    diag[:],
                                    start=True,
                                    stop=True,
                                    skip_group_check=True,
                                )
                        for u in range(NIT):
                            copy_rot(wT_sb[:, u, :], wt_ps[u][:, :S])
                        # ---- stage 5: AV for this head -> attnT
                        av_ps = psum("avps")
                        for u in range(NIT):
                            nc.tensor.matmul(
                                av_ps[hp : hp + 64, :S],
                                v_sb[:, u, h * DH : (h + 1) * DH],
                                wT_sb[:, u, :],
                                start=(u == 0),
                                stop=(u == NIT - 1),
                            )
                        copy_rot(
                            attnT_sb[hp : hp + 64, ho, :], av_ps[hp : hp + 64, :S]
                        )

                # software pipeline: keep PE fed while head-pair bounces land
                stage3(0)
                for pair in range(6):
                    if pair + 1 < 6:
                        stage3(pair + 1)
                    stage45(pair)

                # ---------- stage 6: output projection --------------------
                w_sb = wo_sb
                for io in range(NIT):
                    for no in range(2):
                        ps_t = psum("ps")
                        for ko in range(NDT):
                            nc.tensor.matmul(
                                ps_t[:, :384],
                                attnT_sb[:, ko, ts(io, 128)],
                                w_sb[:, ko, ts(no, 384)],
                                start=(ko == 0),
                                stop=(ko == NDT - 1),
                            )
                        o_sb = workp.tile([128, 384], f32, tag="osb")
                        nc.vector.tensor_tensor(
                            o_sb[:],
                            ps_t[:, :384],
                            bor[:, ts(no, 384)],
                            ADD,
                        )
                        nc.sync.dma_start(
                            out[ts(io, 128), ts(no, 384)], o_sb[:]
                        )

    nc.finalize()
    return nc


def _get_program(mm_dtype_name):
    key = ("nc", mm_dtype_name)
    if key not in _CACHE:
        _CACHE[key] = _build(mm_dtype_name)
    return _CACHE[key]


def _np_dtype(mm_dtype_name):
    if mm_dtype_name == "bfloat16":
        import ml_dtypes

        return np.dtype(ml_dtypes.bfloat16)
    if mm_dtype_name == "float16":
        return np.dtype(np.float16)
    return np.dtype(np.float32)


def _host_prep(inputs, mm_dtype_name):
    f = np.float32
    cdt = _np_dtype(mm_dtype_name)
    x = np.asarray(inputs["x"], f)
    rel = np.asarray(inputs["rel_pos_emb"], f)
    rel_used = rel[MAX_POS - S : MAX_POS - S + NP]          # rows 128..894

    def conv(a):
        return np.ascontiguousarray(np.asarray(a, f).astype(cdt))

    base = {
        "wq": conv(np.asarray(inputs["Wq"], f) * SCALE),
        "wk": conv(inputs["Wk"]),
        "wv": conv(inputs["Wv"]),
        "wpk": conv(inputs["Wpk"]),
        "wpq": conv(np.asarray(inputs["Wpq"], f) * SCALE),
        "wo": conv(inputs["Wo"]),
        "bq": ((np.asarray(inputs["bq"], f) + np.asarray(inputs["q_bias"], f))
               * SCALE).astype(f),
        "bk": np.asarray(inputs["bk"], f),
        "bv": (np.asarray(inputs["bv"], f) + np.asarray(inputs["v_bias"], f)
               ).astype(f),
        "bo": np.asarray(inputs["bo"], f),
        "relkT": conv(np.pad(rel_used.T, ((0, 0), (0, 1)))),
    }
    in_maps = []
    for b in range(B):
        m = dict(base)
        m["xT"] = conv(x[b].T)
        in_maps.append(m)
    return in_maps


def _get_runner():
    """Build (once) a jitted SPMD executor for the compiled program.

    Mirrors concourse.bass2jax.run_bass_via_pjrt's multi-core path but caches
    the jitted callable so repeated kernel() calls don't re-trace/re-compile.
    """
    key = "runner"
    if key in _CACHE:
        return _CACHE[key]
    _import_concourse()
    import jax
    import jax.numpy as jnp  # noqa: F401
    from jax.sharding import Mesh, PartitionSpec
    from jax.experimental.shard_map import shard_map
    import concourse.mybir as mybir
    from concourse import bass2jax

    mm_dtype = os.environ.get("BASS_MM_DTYPE", "bfloat16")
    nc = _get_program(mm_dtype)
    bass2jax.install_neuronx_cc_hook()

    partition_name = (
        nc.partition_id_tensor.name if nc.partition_id_tensor else None
    )
    in_names, out_names, out_avals, zero_outs = [], [], [], []
    for alloc in nc.m.functions[0].allocations:
        if not isinstance(alloc, mybir.MemoryLocationSet):
            continue
        name = alloc.memorylocations[0].name
        if alloc.kind == "ExternalInput":
            if name != partition_name:
                in_names.append(name)
        elif alloc.kind == "ExternalOutput":
            out_names.append(name)
            shape = tuple(alloc.tensor_shape)
            dtype = mybir.dt.np(alloc.dtype)
            out_avals.append(jax.core.ShapedArray(shape, dtype))
            zero_outs.append(np.zeros(shape, dtype))
    n_params = len(in_names)
    all_names = in_names + out_names
    if partition_name is not None:
        all_names = all_names + [partition_name]

    def _body(*args):
        operands = list(args)
        if partition_name is not None:
            operands.append(bass2jax.partition_id_tensor())
        outs = bass2jax._bass_exec_p.bind(
            *operands,
            out_avals=tuple(out_avals),
            in_names=tuple(all_names),
            out_names=tuple(out_names),
            lowering_input_output_aliases=(),
            sim_require_finite=True,
            sim_require_nnan=True,
            nc=nc,
        )
        return tuple(outs)

    devices = jax.devices()[:B]
    mesh = Mesh(np.asarray(devices), ("core",))
    n_outs = len(out_names)
    sharded = jax.jit(
        shard_map(
            _body,
            mesh=mesh,
            in_specs=(PartitionSpec("core"),) * (n_params + n_outs),
            out_specs=(PartitionSpec("core"),) * n_outs,
            check_rep=False,
        ),
        donate_argnums=tuple(range(n_params, n_params + n_outs)),
        keep_unused=True,
    )

    def run(in_maps):
        concat_in = [
            np.concatenate([np.asarray(in_maps[c][nm]) for c in range(B)], axis=0)
            for nm in in_names
        ]
        concat_zeros = [
            np.zeros((B * z.shape[0], *z.shape[1:]), z.dtype) for z in zero_outs
        ]
        out_arrs = sharded(*concat_in, *concat_zeros)
        return [
            {
                nm: np.asarray(out_arrs[i]).reshape(B, *out_avals[i].shape)[c]
                for i, nm in enumerate(out_names)
            }
            for c in range(B)
        ]

    _CACHE[key] = run
    return run


def _run(inputs, trace=False):
    run = _get_runner()
    in_maps = _host_prep(inputs, os.environ.get("BASS_MM_DTYPE", "bfloat16"))
    results = run(in_maps)
    outs = np.stack([np.asarray(results[b]["out"]) for b in range(B)])
    return outs.astype(np.float32), None


def kernel(**inputs) -> np.ndarray:
    out, _ = _run(inputs)
    return out
